# revision 24
# baseline (speedup 1.0000x reference)
"""Trainium2 Bass kernel for nn_Conv_block_57690000720236.

Reference computation (per batch image b):
  - 3x3 SAME conv "high" branch: 64ch -> 64ch
  - low branch: 3x3 conv 64ch -> 16ch, then 1x1 conv 16ch -> 64ch
  - output position (b,y,x) takes the high value if its flat index is in
    mask_idx, the low value if in inv_mask_idx (inv wins on overlap), 0 if
    in neither.

Strategy (8 NeuronCores, data-parallel over batch):
  - Core b computes BOTH branches densely for image b; the low branch is
    folded on the host (W_low = w2 @ w1) so both branches are 3x3 convs,
    evaluated together as M=128 output columns (64 high + 64 low).
  - 5 matmul passes per 4-row chunk (N=512 positions), the minimum for a
    K=576 contraction on a 128-row PE:
      pass 1-3: tap pairs (ky0,c)+(ky1,c), K=128, from reg0 whose partition
                halves hold (P | P shifted down 1 row)
      pass 4:   tap pair (ky2,kx0)+(ky2,kx1), K=128, from reg1 whose halves
                hold rows+2 of (P | P shifted left 1 col)
      pass 5:   tap (ky2,kx2), K=64, from reg1 partitions 0-63
  - Routing: the scalar engine evicts the low half (PSUM partitions 64-127)
    to SBUF, an on-chip DMA moves it across partitions into the output
    buffer (partitions 0-63), then one DVE copy_predicated overwrites with
    the high half where the host-built mask routes high. This keeps the PE
    instruction stream free of stalls (the baseline's identity-matmul merge
    made the PE wait on DVE every chunk, which kept the HAM clock gate at
    1.2 GHz; with a pure conv stream the PE runs warm at 2.4 GHz).
"""

import numpy as np
import ml_dtypes

import concourse.bacc as bacc
import concourse.mybir as mybir
import concourse.tile as tile
from concourse.bass_utils import run_bass_kernel_spmd

B, CIN, H, W = 8, 64, 128, 128
COUT, KER = 64, 3
NPOS = H * W                 # 16384 positions per core
WP = W + 2                   # padded row length 130
N_TILES = 8                  # image row-tiles
TROWS = H // N_TILES         # 16 output rows per tile
CHUNK_ROWS = 4               # output rows per matmul chunk
CHUNK = CHUNK_ROWS * W       # 512 positions per chunk
CHUNKS_PER_TILE = TROWS // CHUNK_ROWS
TILE_POS = TROWS * W         # 2048 positions per tile
RC = TROWS * WP              # region cols per tile (16 rows x 130)
F32 = mybir.dt.float32
BF16 = mybir.dt.bfloat16
U8 = mybir.dt.uint8
OUTDT = mybir.dt.bfloat16    # on-chip merge + writeback dtype
WBLK = 5 * 128               # weight blob: 5 matmul blocks


def _build_program(need_zero_fix: bool):
    nc = bacc.Bacc("TRN2", target_bir_lowering=False, debug=False, num_devices=B)

    r0_d = nc.dram_tensor("reg0", [N_TILES, 128, RC], BF16, kind="ExternalInput")
    r1_d = nc.dram_tensor("reg1", [N_TILES, 128, RC], BF16, kind="ExternalInput")
    w_d = nc.dram_tensor("wblob", [128, WBLK], BF16, kind="ExternalInput")
    m_d = nc.dram_tensor("mhigh", [COUT, NPOS], U8, kind="ExternalInput")
    if need_zero_fix:
        mz_d = nc.dram_tensor("mzero", [COUT, NPOS], U8, kind="ExternalInput")
    out_d = nc.dram_tensor("out", [COUT, NPOS], OUTDT, kind="ExternalOutput")

    with tile.TileContext(nc) as tc:
        with (
            tc.tile_pool(name="const", bufs=1) as cpool,
            tc.tile_pool(name="outp", bufs=2) as opool,
            tc.tile_pool(name="lowp", bufs=3) as lpool,
            tc.tile_pool(name="psum", bufs=7, space="PSUM") as pspool,
        ):
            # DMA issue is spread across sequencer queues so the per-chunk
            # low-move DMAs (gpsimd) never sit behind the input loads (sync/
            # scalar) — a shared queue here serialized the merge pipeline and
            # stalled the PE for ~15us mid-kernel.
            wt = cpool.tile([128, WBLK], BF16, tag="wblob")
            nc.sync.dma_start(wt[:], w_d[:])
            mt = cpool.tile([COUT, NPOS], U8, tag="mhigh")
            if need_zero_fix:
                mzt = cpool.tile([COUT, NPOS], U8, tag="mzero")
                nc.gpsimd.dma_start(mzt[:], mz_d[:])
                zt = cpool.tile([COUT, CHUNK], OUTDT, tag="zeros")
                nc.any.memset(zt[:], 0.0)

            # Input loads are issued just-in-time, two tiles ahead of the
            # compute loop: queueing all 8.5MB upfront made every per-chunk
            # merge DMA wait ~20us behind the bulk stream (shared rings).
            r0t = cpool.tile([128, N_TILES * RC], BF16, tag="reg0")
            r1t = cpool.tile([128, N_TILES * RC], BF16, tag="reg1")

            def load_tile(i):
                nc.sync.dma_start(r0t[:, i * RC:(i + 1) * RC], r0_d[i])
                nc.sync.dma_start(r1t[:, i * RC:(i + 1) * RC], r1_d[i])

            load_tile(0)
            load_tile(1)
            nc.sync.dma_start(mt[:], m_d[:])
            v0 = r0t[:].rearrange("p (t r x) -> p t r x", r=TROWS, x=WP)
            v1 = r1t[:].rearrange("p (t r x) -> p t r x", r=TROWS, x=WP)

            for i in range(N_TILES):
                if i + 2 < N_TILES:
                    load_tile(i + 2)
                out_sb = opool.tile([COUT, TILE_POS], OUTDT, tag="osb")
                for j in range(CHUNKS_PER_TILE):
                    l0 = j * CHUNK_ROWS
                    so = j * CHUNK
                    s = i * TILE_POS + so

                    pt = pspool.tile([128, CHUNK], F32, tag="acc")
                    pv = pt[:].rearrange("p (r x) -> p r x", x=W)

                    for c in range(3):
                        nc.tensor.matmul(
                            pv,
                            wt[:, c * 128:(c + 1) * 128],
                            v0[:, i, l0:l0 + CHUNK_ROWS, c:c + W],
                            start=(c == 0),
                            stop=False,
                        )
                    nc.tensor.matmul(
                        pv,
                        wt[:, 3 * 128:4 * 128],
                        v1[:, i, l0:l0 + CHUNK_ROWS, 0:W],
                        start=False,
                        stop=False,
                    )
                    nc.tensor.matmul(
                        pv,
                        wt[0:64, 4 * 128:5 * 128],
                        v1[0:64, i, l0:l0 + CHUNK_ROWS, 2:2 + W],
                        start=False,
                        stop=True,
                    )

                    # low half -> SBUF staging (ACT, casting to bf16), then
                    # across partitions into out_sb (gpsimd-issued DMA, kept
                    # off the sync ring that carries the bulk input loads),
                    # then the high half overlays where the mask routes high
                    lowt = lpool.tile([128, CHUNK], OUTDT, tag="low")
                    nc.scalar.copy(lowt[64:128, :], pt[64:128, :])
                    nc.gpsimd.dma_start(
                        out_sb[:, so:so + CHUNK], lowt[64:128, :]
                    )
                    nc.vector.copy_predicated(
                        out_sb[:, so:so + CHUNK], mt[:, s:s + CHUNK], pt[0:64, :]
                    )
                    if need_zero_fix:
                        nc.vector.copy_predicated(
                            out_sb[:, so:so + CHUNK], mzt[:, s:s + CHUNK], zt[:]
                        )

                nc.scalar.dma_start(
                    out_d[:, i * TILE_POS:(i + 1) * TILE_POS], out_sb[:]
                )

    nc.compile()
    return nc


def _prepare_host(inx, mask_idx, inv_mask_idx, high_w, low1_w, low2_w):
    inx = np.asarray(inx, dtype=np.float32)
    mask_idx = np.asarray(mask_idx).astype(np.int64)
    inv_mask_idx = np.asarray(inv_mask_idx).astype(np.int64)
    high_w = np.asarray(high_w, dtype=np.float32)
    low1_w = np.asarray(low1_w, dtype=np.float32)
    low2_w = np.asarray(low2_w, dtype=np.float32)

    # zero-padded images P [B, 64, 130, 130]
    inxp = np.zeros((B, CIN, H + 2, WP), np.float32)
    inxp[:, :, 1:-1, 1:-1] = inx

    # reg0: halves (P rows r0..r0+15 | P rows r0+1..r0+16)
    # reg1: halves (P rows r0+2..r0+17 | same shifted left one col)
    reg0 = np.zeros((B, N_TILES, 128, TROWS, WP), ml_dtypes.bfloat16)
    reg1 = np.zeros((B, N_TILES, 128, TROWS, WP), ml_dtypes.bfloat16)
    for i in range(N_TILES):
        r0 = i * TROWS
        reg0[:, i, 0:64] = inxp[:, :, r0:r0 + TROWS]
        reg0[:, i, 64:128] = inxp[:, :, r0 + 1:r0 + 1 + TROWS]
        reg1[:, i, 0:64] = inxp[:, :, r0 + 2:r0 + 2 + TROWS]
        reg1[:, i, 64:128, :, 0:WP - 1] = inxp[:, :, r0 + 2:r0 + 2 + TROWS, 1:]
    reg0 = reg0.reshape(B, N_TILES, 128, RC)
    reg1 = reg1.reshape(B, N_TILES, 128, RC)

    # fold the low branch: W_low[o, c, ky, kx] = sum_m w2[o, m] w1[m, c, ky, kx]
    w2 = low2_w.reshape(COUT, -1).astype(np.float64)
    wl = np.einsum("om,mckl->ockl", w2, low1_w.astype(np.float64)).astype(np.float32)
    wh = high_w

    # weight blob [128, 5*128] bf16; lhsT[k, m], m = output col (0-63 high,
    # 64-127 low-folded); k partition halves match the reg layouts above
    blob = np.zeros((128, WBLK), ml_dtypes.bfloat16)
    for c in range(3):
        blk = blob[:, c * 128:(c + 1) * 128]
        blk[0:64, 0:64] = wh[:, :, 0, c].T
        blk[0:64, 64:128] = wl[:, :, 0, c].T
        blk[64:128, 0:64] = wh[:, :, 1, c].T
        blk[64:128, 64:128] = wl[:, :, 1, c].T
    b3 = blob[:, 3 * 128:4 * 128]
    b3[0:64, 0:64] = wh[:, :, 2, 0].T
    b3[0:64, 64:128] = wl[:, :, 2, 0].T
    b3[64:128, 0:64] = wh[:, :, 2, 1].T
    b3[64:128, 64:128] = wl[:, :, 2, 1].T
    b4 = blob[:, 4 * 128:5 * 128]
    b4[0:64, 0:64] = wh[:, :, 2, 2].T
    b4[0:64, 64:128] = wl[:, :, 2, 2].T

    ntotal = B * NPOS
    in_mask = np.zeros(ntotal, dtype=bool)
    in_inv = np.zeros(ntotal, dtype=bool)
    in_mask[mask_idx] = True
    in_inv[inv_mask_idx] = True
    # high wins only where inv doesn't claim (reference scatters inv last)
    m_high = in_mask & ~in_inv
    neither = ~(in_mask | in_inv)
    need_zero_fix = bool(neither.any())

    in_maps = []
    for b in range(B):
        sl = slice(b * NPOS, (b + 1) * NPOS)
        mh = np.ascontiguousarray(
            np.broadcast_to(m_high[sl].astype(np.uint8)[None, :], (COUT, NPOS))
        )
        m = {"reg0": reg0[b], "reg1": reg1[b], "wblob": blob, "mhigh": mh}
        if need_zero_fix:
            m["mzero"] = np.ascontiguousarray(
                np.broadcast_to(neither[sl].astype(np.uint8)[None, :], (COUT, NPOS))
            )
        in_maps.append(m)
    return in_maps, need_zero_fix


def _run(inputs: dict, trace: bool = False):
    in_maps, need_zero_fix = _prepare_host(**inputs)
    nc = _build_program(need_zero_fix)
    res = run_bass_kernel_spmd(nc, in_maps, list(range(B)), trace=trace)
    out = np.stack(
        [np.asarray(res.results[b]["out"]).astype(np.float32).reshape(COUT, H, W)
         for b in range(B)]
    ).astype(np.float32)
    return out, res


def kernel(**inputs) -> np.ndarray:
    out, _ = _run(inputs, trace=False)
    return out


# revision 27
# speedup vs baseline: 1.1712x; 1.1712x over previous
"""Trainium2 Bass kernel for nn_Conv_block_57690000720236.

Reference computation (per batch image b):
  - 3x3 SAME conv "high" branch: 64ch -> 64ch
  - low branch: 3x3 conv 64ch -> 16ch, then 1x1 conv 16ch -> 64ch
  - output position (b,y,x) takes the high value if its flat index is in
    mask_idx, the low value if in inv_mask_idx (inv wins on overlap), 0 if
    in neither.

Strategy (8 NeuronCores, data-parallel over batch):
  - Core b computes BOTH branches densely for image b; the low branch is
    folded on the host (W_low = w2 @ w1) so both branches are 3x3 convs,
    evaluated together as M=128 output columns (64 high + 64 low).
  - 5 matmul passes per 4-row chunk (N=512 positions), the minimum for a
    K=576 contraction on a 128-row PE:
      pass 1-3: tap pairs (ky0,c)+(ky1,c), K=128, from reg0 whose partition
                halves hold (P | P shifted down 1 row)
      pass 4:   tap pair (ky2,kx0)+(ky2,kx1), K=128, from reg1 whose halves
                hold rows+2 of (P | P shifted left 1 col)
      pass 5:   tap (ky2,kx2), K=64, from reg1 partitions 0-63
  - Routing: the scalar engine evicts the low half (PSUM partitions 64-127)
    to SBUF, an on-chip DMA moves it across partitions into the output
    buffer (partitions 0-63), then one DVE copy_predicated overwrites with
    the high half where the host-built mask routes high. This keeps the PE
    instruction stream free of stalls (the baseline's identity-matmul merge
    made the PE wait on DVE every chunk, which kept the HAM clock gate at
    1.2 GHz; with a pure conv stream the PE runs warm at 2.4 GHz).
"""

import numpy as np
import ml_dtypes

import concourse.bacc as bacc
import concourse.mybir as mybir
import concourse.tile as tile
from concourse.bass_utils import run_bass_kernel_spmd

B, CIN, H, W = 8, 64, 128, 128
COUT, KER = 64, 3
NPOS = H * W                 # 16384 positions per core
WP = W + 2                   # padded row length 130
N_TILES = 8                  # image row-tiles
TROWS = H // N_TILES         # 16 output rows per tile
CHUNK_ROWS = 4               # output rows per matmul chunk
CHUNK = CHUNK_ROWS * W       # 512 positions per chunk
CHUNKS_PER_TILE = TROWS // CHUNK_ROWS
TILE_POS = TROWS * W         # 2048 positions per tile
RC = TROWS * WP              # region cols per tile (16 rows x 130)
F32 = mybir.dt.float32
BF16 = mybir.dt.bfloat16
U8 = mybir.dt.uint8
OUTDT = mybir.dt.bfloat16    # on-chip merge + writeback dtype
WBLK = 5 * 128               # weight blob: 5 matmul blocks


def _build_program(need_zero_fix: bool):
    nc = bacc.Bacc("TRN2", target_bir_lowering=False, debug=False, num_devices=B)

    r0_d = nc.dram_tensor("reg0", [N_TILES, 128, RC], BF16, kind="ExternalInput")
    r1_d = nc.dram_tensor("reg1", [N_TILES, 128, RC], BF16, kind="ExternalInput")
    w_d = nc.dram_tensor("wblob", [128, WBLK], BF16, kind="ExternalInput")
    m_d = nc.dram_tensor("mhigh", [COUT, NPOS], U8, kind="ExternalInput")
    if need_zero_fix:
        mz_d = nc.dram_tensor("mzero", [COUT, NPOS], U8, kind="ExternalInput")
    out_d = nc.dram_tensor("out", [COUT, NPOS], OUTDT, kind="ExternalOutput")

    with tile.TileContext(nc) as tc:
        with (
            tc.tile_pool(name="const", bufs=1) as cpool,
            tc.tile_pool(name="outp", bufs=2) as opool,
            tc.tile_pool(name="evp", bufs=4) as epool,
            tc.tile_pool(name="psum", bufs=5, space="PSUM") as pspool,
            tc.tile_pool(name="psumw", bufs=1, space="PSUM") as pwpool,
        ):
            # DMA issue is spread across sequencer queues so the per-chunk
            # low-move DMAs (gpsimd) never sit behind the input loads (sync/
            # scalar) — a shared queue here serialized the merge pipeline and
            # stalled the PE for ~15us mid-kernel.
            wt = cpool.tile([128, WBLK], BF16, tag="wblob")
            nc.sync.dma_start(wt[:], w_d[:])
            mt = cpool.tile([COUT, NPOS], U8, tag="mhigh")
            if need_zero_fix:
                mzt = cpool.tile([COUT, NPOS], U8, tag="mzero")
                nc.gpsimd.dma_start(mzt[:], mz_d[:])
                zt = cpool.tile([COUT, CHUNK], OUTDT, tag="zeros")
                nc.any.memset(zt[:], 0.0)

            # Input loads are issued just-in-time, two tiles ahead of the
            # compute loop: queueing all 8.5MB upfront made every per-chunk
            # merge DMA wait ~20us behind the bulk stream (shared rings).
            r0t = cpool.tile([128, N_TILES * RC], BF16, tag="reg0")
            r1t = cpool.tile([128, N_TILES * RC], BF16, tag="reg1")

            def load_tile(i):
                nc.sync.dma_start(r0t[:, i * RC:(i + 1) * RC], r0_d[i])
                nc.sync.dma_start(r1t[:, i * RC:(i + 1) * RC], r1_d[i])

            load_tile(0)
            load_tile(1)
            nc.sync.dma_start(mt[:], m_d[:])
            v0 = r0t[:].rearrange("p (t r x) -> p t r x", r=TROWS, x=WP)
            v1 = r1t[:].rearrange("p (t r x) -> p t r x", r=TROWS, x=WP)

            # Warm-up matmuls on dummy data while the first input tiles are
            # still in flight: the PE HAM clock gate needs ~3.4us of
            # sustained activity to lift the 1.2GHz cold throttle, so burn
            # that window before the real conv stream begins.
            dummy = cpool.tile([128, CHUNK], BF16, tag="dummy")
            nc.vector.memset(dummy[:], 0.0)
            warmp = pwpool.tile([128, CHUNK], F32, tag="warm")
            for _ in range(16):
                nc.tensor.matmul(
                    warmp[:], dummy[:, 0:128], dummy[:], start=True, stop=True
                )

            for i in range(N_TILES):
                if i + 2 < N_TILES:
                    load_tile(i + 2)
                out_sb = opool.tile([COUT, TILE_POS], OUTDT, tag="osb")
                for j in range(CHUNKS_PER_TILE):
                    l0 = j * CHUNK_ROWS
                    so = j * CHUNK
                    s = i * TILE_POS + so

                    pt = pspool.tile([128, CHUNK], F32, tag="acc")
                    pv = pt[:].rearrange("p (r x) -> p r x", x=W)

                    for c in range(3):
                        nc.tensor.matmul(
                            pv,
                            wt[:, c * 128:(c + 1) * 128],
                            v0[:, i, l0:l0 + CHUNK_ROWS, c:c + W],
                            start=(c == 0),
                            stop=False,
                        )
                    nc.tensor.matmul(
                        pv,
                        wt[:, 3 * 128:4 * 128],
                        v1[:, i, l0:l0 + CHUNK_ROWS, 0:W],
                        start=False,
                        stop=False,
                    )
                    nc.tensor.matmul(
                        pv,
                        wt[0:64, 4 * 128:5 * 128],
                        v1[0:64, i, l0:l0 + CHUNK_ROWS, 2:2 + W],
                        start=False,
                        stop=True,
                    )

                    # One ACT copy evicts the whole 128-partition chunk to
                    # SBUF (bank freed ~0.7us after MM5, so PSUM never backs
                    # up into the PE). The merge then runs on SBUF tiles:
                    # DMA moves the low half across partitions into out_sb,
                    # and copy_predicated overlays the high half per mask.
                    ev = epool.tile([128, CHUNK], OUTDT, tag="ev")
                    nc.scalar.copy(ev[:], pt[:])
                    nc.sync.dma_start(
                        out_sb[:, so:so + CHUNK], ev[64:128, :]
                    )
                    nc.vector.copy_predicated(
                        out_sb[:, so:so + CHUNK], mt[:, s:s + CHUNK], ev[0:64, :]
                    )
                    if need_zero_fix:
                        nc.vector.copy_predicated(
                            out_sb[:, so:so + CHUNK], mzt[:, s:s + CHUNK], zt[:]
                        )

                nc.scalar.dma_start(
                    out_d[:, i * TILE_POS:(i + 1) * TILE_POS], out_sb[:]
                )

    nc.compile()
    return nc


def _prepare_host(inx, mask_idx, inv_mask_idx, high_w, low1_w, low2_w):
    inx = np.asarray(inx, dtype=np.float32)
    mask_idx = np.asarray(mask_idx).astype(np.int64)
    inv_mask_idx = np.asarray(inv_mask_idx).astype(np.int64)
    high_w = np.asarray(high_w, dtype=np.float32)
    low1_w = np.asarray(low1_w, dtype=np.float32)
    low2_w = np.asarray(low2_w, dtype=np.float32)

    # zero-padded images P [B, 64, 130, 130]
    inxp = np.zeros((B, CIN, H + 2, WP), np.float32)
    inxp[:, :, 1:-1, 1:-1] = inx

    # reg0: halves (P rows r0..r0+15 | P rows r0+1..r0+16)
    # reg1: halves (P rows r0+2..r0+17 | same shifted left one col)
    reg0 = np.zeros((B, N_TILES, 128, TROWS, WP), ml_dtypes.bfloat16)
    reg1 = np.zeros((B, N_TILES, 128, TROWS, WP), ml_dtypes.bfloat16)
    for i in range(N_TILES):
        r0 = i * TROWS
        reg0[:, i, 0:64] = inxp[:, :, r0:r0 + TROWS]
        reg0[:, i, 64:128] = inxp[:, :, r0 + 1:r0 + 1 + TROWS]
        reg1[:, i, 0:64] = inxp[:, :, r0 + 2:r0 + 2 + TROWS]
        reg1[:, i, 64:128, :, 0:WP - 1] = inxp[:, :, r0 + 2:r0 + 2 + TROWS, 1:]
    reg0 = reg0.reshape(B, N_TILES, 128, RC)
    reg1 = reg1.reshape(B, N_TILES, 128, RC)

    # fold the low branch: W_low[o, c, ky, kx] = sum_m w2[o, m] w1[m, c, ky, kx]
    w2 = low2_w.reshape(COUT, -1).astype(np.float64)
    wl = np.einsum("om,mckl->ockl", w2, low1_w.astype(np.float64)).astype(np.float32)
    wh = high_w

    # weight blob [128, 5*128] bf16; lhsT[k, m], m = output col (0-63 high,
    # 64-127 low-folded); k partition halves match the reg layouts above
    blob = np.zeros((128, WBLK), ml_dtypes.bfloat16)
    for c in range(3):
        blk = blob[:, c * 128:(c + 1) * 128]
        blk[0:64, 0:64] = wh[:, :, 0, c].T
        blk[0:64, 64:128] = wl[:, :, 0, c].T
        blk[64:128, 0:64] = wh[:, :, 1, c].T
        blk[64:128, 64:128] = wl[:, :, 1, c].T
    b3 = blob[:, 3 * 128:4 * 128]
    b3[0:64, 0:64] = wh[:, :, 2, 0].T
    b3[0:64, 64:128] = wl[:, :, 2, 0].T
    b3[64:128, 0:64] = wh[:, :, 2, 1].T
    b3[64:128, 64:128] = wl[:, :, 2, 1].T
    b4 = blob[:, 4 * 128:5 * 128]
    b4[0:64, 0:64] = wh[:, :, 2, 2].T
    b4[0:64, 64:128] = wl[:, :, 2, 2].T

    ntotal = B * NPOS
    in_mask = np.zeros(ntotal, dtype=bool)
    in_inv = np.zeros(ntotal, dtype=bool)
    in_mask[mask_idx] = True
    in_inv[inv_mask_idx] = True
    # high wins only where inv doesn't claim (reference scatters inv last)
    m_high = in_mask & ~in_inv
    neither = ~(in_mask | in_inv)
    need_zero_fix = bool(neither.any())

    in_maps = []
    for b in range(B):
        sl = slice(b * NPOS, (b + 1) * NPOS)
        mh = np.ascontiguousarray(
            np.broadcast_to(m_high[sl].astype(np.uint8)[None, :], (COUT, NPOS))
        )
        m = {"reg0": reg0[b], "reg1": reg1[b], "wblob": blob, "mhigh": mh}
        if need_zero_fix:
            m["mzero"] = np.ascontiguousarray(
                np.broadcast_to(neither[sl].astype(np.uint8)[None, :], (COUT, NPOS))
            )
        in_maps.append(m)
    return in_maps, need_zero_fix


def _run(inputs: dict, trace: bool = False):
    in_maps, need_zero_fix = _prepare_host(**inputs)
    nc = _build_program(need_zero_fix)
    res = run_bass_kernel_spmd(nc, in_maps, list(range(B)), trace=trace)
    out = np.stack(
        [np.asarray(res.results[b]["out"]).astype(np.float32).reshape(COUT, H, W)
         for b in range(B)]
    ).astype(np.float32)
    return out, res


def kernel(**inputs) -> np.ndarray:
    out, _ = _run(inputs, trace=False)
    return out
